# revision 1
# baseline (speedup 1.0000x reference)
"""Trainium2 Bass kernel for the DfOp deep-filtering module.

out[b, t, f<96]  = sum_{k=0..4} coefs[b, k, t, f] (*) spec[b, t-4+k, f]   (complex mult)
out[b, t, f>=96] = spec[b, t, f]                                          (passthrough)

Sharding: data-parallel over batch B=8 -> one batch element per NeuronCore.

Per-core layout: partition p in [0,128) holds the 32-timestep block
t in [32p, 32p+32); the free dimension is (t-within-block, freq, re/im)
interleaved exactly as in DRAM.  A 4-slot halo (previous block's last 4
timesteps) is prepended in the free dim, so the causal 5-tap window is a pure
free-dim offset: tap k of output slot i reads free slot (i + k).  All DMA is
contiguous runs of >=768B.

Compute (all fp32, bit-accurate accumulation):
  DVE: 4 real products per tap (rr, -ii, ri, ir) via strided access patterns
       (fp32 tensor_tensor is 1x-mode regardless of stride, so the strided
       deinterleaving views cost nothing extra).  The -ii negation is fused
       into the product via scalar_tensor_tensor.
  PE : accumulates the 20 products of each chunk into PSUM with
       identity-weight matmuls (PSUM fp32 accumulate).
  ACT: interleaves the PSUM re/im planes into the output tile.
  DMA: hi-band passthrough goes DRAM->DRAM, never touching SBUF.
"""

import numpy as np

import concourse.bacc as bacc
import concourse.mybir as mybir
from concourse.tile import TileContext
from concourse.bass_utils import run_bass_kernel_spmd

B = 8          # batch / cores
T = 4096       # time steps
F = 481        # total freq bins
NF = 96        # deep-filtered freq bins
FS = 5         # frame size (causal taps)
ROW = 2 * F    # floats per DRAM time row        (962)
U = 2 * NF     # lo-band floats per time row     (192)
P = 128        # partitions
TB = T // P    # timesteps per partition block   (32)
TI = 8         # timesteps per chunk
NCH = TB // TI # chunks                          (4)
CW = TI * NF   # product/psum cols per chunk     (768)
CWU = TI * U   # out-tile cols per chunk         (1536)
JH = TB + FS - 1  # haloed time slots per row    (36)

_nc_cache = None


def _body(nc, tc, spec_d, coefs_d, ident_d, out_d):
    f32 = mybir.dt.float32
    mult = mybir.AluOpType.mult

    specv = spec_d.rearrange("(q i) u -> q i u", i=TB)   # [128, 32, 962]
    outv = out_d.rearrange("(q i) u -> q i u", i=TB)     # [128, 32, 962]

    with (
        tc.tile_pool(name="const", bufs=1) as cpool,
        tc.tile_pool(name="spec", bufs=1) as spool,
        tc.tile_pool(name="coef", bufs=8) as kpool,
        tc.tile_pool(name="prod", bufs=6) as ppool,
        tc.tile_pool(name="outp", bufs=3) as opool,
        tc.tile_pool(name="psum", bufs=2, space="PSUM") as pspool,
    ):
        ident_sb = cpool.tile([P, P], f32)
        nc.sync.dma_start(out=ident_sb[:], in_=ident_d)

        # hi-band passthrough: DRAM -> DRAM, split for pipelining
        for c in range(4):
            r0, r1 = c * (T // 4), (c + 1) * (T // 4)
            nc.sync.dma_start(out=out_d[r0:r1, U:ROW], in_=spec_d[r0:r1, U:ROW])

        # lo-band spec, haloed: row p free slot j in [0,36) holds t = 32p-4+j
        spec_raw = spool.tile([P, JH * U], f32)
        sv = spec_raw[:].rearrange("p (j u) -> p j u", u=U)      # [128, 36, 192]
        nc.vector.memzero(sv[0:1, 0:FS - 1, :])                   # t<0 -> 0
        nc.sync.dma_start(out=sv[:, FS - 1:JH, :], in_=specv[:, :, 0:U])
        nc.sync.dma_start(
            out=sv[1:P, 0:FS - 1, :],
            in_=specv[0:P - 1, TB - (FS - 1):TB, 0:U],
        )
        svfc = spec_raw[:].rearrange("p (j f c) -> p j f c", f=NF, c=2)

        for ch in range(NCH):
            i0 = ch * TI
            ctiles = []
            for k in range(FS):
                ct = kpool.tile([P, TI * U], f32, tag="coef")
                src = coefs_d[k].rearrange("(q i) u -> q i u", i=TB)[:, i0:i0 + TI, :]
                nc.sync.dma_start(
                    out=ct[:].rearrange("p (i u) -> p i u", u=U), in_=src
                )
                ctiles.append(ct)

            ps_re = pspool.tile([P, CW], f32, tag="psre")
            ps_im = pspool.tile([P, CW], f32, tag="psim")

            for k in range(FS):
                s_re = svfc[:, i0 + k:i0 + k + TI, :, 0]          # [128, 8, 96]
                s_im = svfc[:, i0 + k:i0 + k + TI, :, 1]
                cvfc = ctiles[k][:].rearrange("p (i f c) -> p i f c", f=NF, c=2)
                c_re = cvfc[:, :, :, 0]
                c_im = cvfc[:, :, :, 1]

                prr = ppool.tile([P, CW], f32, tag="prod")
                pii = ppool.tile([P, CW], f32, tag="prod")
                pri = ppool.tile([P, CW], f32, tag="prod")
                pir = ppool.tile([P, CW], f32, tag="prod")
                pv = lambda t: t[:].rearrange("p (i f) -> p i f", f=NF)

                nc.vector.tensor_mul(out=pv(prr), in0=s_re, in1=c_re)
                nc.vector.scalar_tensor_tensor(
                    out=pv(pii), in0=s_im, scalar=-1.0, in1=c_im,
                    op0=mult, op1=mult,
                )
                nc.vector.tensor_mul(out=pv(pri), in0=s_re, in1=c_im)
                nc.vector.tensor_mul(out=pv(pir), in0=s_im, in1=c_re)

                # PSUM accumulate via identity matmul:
                #   ps_re = sum_k (rr_k - ii_k),  ps_im = sum_k (ri_k + ir_k)
                for prod, ps, first, last in (
                    (prr, ps_re, k == 0, False),
                    (pii, ps_re, False, k == FS - 1),
                    (pri, ps_im, k == 0, False),
                    (pir, ps_im, False, k == FS - 1),
                ):
                    for a, b in ((0, 512), (512, CW)):
                        nc.tensor.matmul(
                            ps[:, a:b], ident_sb[:], prod[:, a:b],
                            start=first, stop=last,
                        )

            out_t = opool.tile([P, CWU], f32, tag="out")
            ov = out_t[:].rearrange("p (i f c) -> p i f c", f=NF, c=2)
            psv = lambda t: t[:].rearrange("p (i f) -> p i f", f=NF)
            nc.scalar.copy(out=ov[:, :, :, 0], in_=psv(ps_re))
            nc.scalar.copy(out=ov[:, :, :, 1], in_=psv(ps_im))
            nc.sync.dma_start(
                out=outv[:, i0:i0 + TI, 0:U],
                in_=out_t[:].rearrange("p (i u) -> p i u", u=U),
            )


def _build_nc():
    nc = bacc.Bacc("TRN2", target_bir_lowering=False, debug=False, num_devices=B)
    f32 = mybir.dt.float32
    spec_d = nc.dram_tensor("spec", [T, ROW], f32, kind="ExternalInput").ap()
    coefs_d = nc.dram_tensor("coefs", [FS, T, U], f32, kind="ExternalInput").ap()
    ident_d = nc.dram_tensor("ident", [P, P], f32, kind="ExternalInput").ap()
    out_d = nc.dram_tensor("out", [T, ROW], f32, kind="ExternalOutput").ap()
    with TileContext(nc) as tc:
        _body(nc, tc, spec_d, coefs_d, ident_d, out_d)
    nc.compile()
    return nc


def _in_maps(spec, coefs):
    spec = np.asarray(spec, dtype=np.float32)
    coefs = np.asarray(coefs, dtype=np.float32)
    ident = np.eye(P, dtype=np.float32)
    maps = []
    for b in range(B):
        maps.append({
            "spec": np.ascontiguousarray(spec[b, 0].reshape(T, ROW)),
            "coefs": np.ascontiguousarray(coefs[b].reshape(FS, T, U)),
            "ident": ident,
        })
    return maps


def kernel(spec, coefs):
    global _nc_cache
    if _nc_cache is None:
        _nc_cache = _build_nc()
    res = run_bass_kernel_spmd(_nc_cache, _in_maps(spec, coefs),
                               core_ids=list(range(B)))
    return np.stack(
        [res.results[b]["out"].reshape(1, T, F, 2) for b in range(B)]
    ).astype(np.float32)


# revision 2
# speedup vs baseline: 1.0210x; 1.0210x over previous
"""Trainium2 Bass kernel for the DfOp deep-filtering module.

out[b, t, f<96]  = sum_{k=0..4} coefs[b, k, t, f] (*) spec[b, t-4+k, f]   (complex mult)
out[b, t, f>=96] = spec[b, t, f]                                          (passthrough)

Sharding: data-parallel over batch B=8 -> one batch element per NeuronCore.

Per-core layout: partition p in [0,128) holds the 32-timestep block
t in [32p, 32p+32); the free dimension is (t-within-block, freq, re/im)
interleaved exactly as in DRAM.  A 4-slot halo (previous block's last 4
timesteps) is prepended in the free dim, so the causal 5-tap window is a pure
free-dim offset: tap k of output slot i reads free slot (i + k).  All DMA is
contiguous runs of >=768B.

Compute (all fp32, bit-accurate accumulation):
  DVE: per tap, 4 real products via strided access patterns (fp32
       tensor_tensor is 1x-mode regardless of stride, so strided
       deinterleaving views cost nothing), with the im*im negation fused via
       scalar_tensor_tensor; then pair-combines D = rr - ii, E = ri + ir.
  PE : accumulates the 5 taps' D (resp. E) into PSUM with identity-weight
       matmuls (exact fp32 PSUM accumulate).
  ACT: interleaves the PSUM re/im planes into the output tile.
  DMA: loads on the Sync HWDGE ring; hi-band DRAM->DRAM passthrough and
       output stores on the Scalar HWDGE ring (independent FIFOs, so the
       12.6MB passthrough never head-blocks the loads).
"""

import numpy as np

import concourse.bacc as bacc
import concourse.mybir as mybir
from concourse.tile import TileContext
from concourse.bass_utils import run_bass_kernel_spmd

B = 8          # batch / cores
T = 4096       # time steps
F = 481        # total freq bins
NF = 96        # deep-filtered freq bins
FS = 5         # frame size (causal taps)
ROW = 2 * F    # floats per DRAM time row        (962)
U = 2 * NF     # lo-band floats per time row     (192)
P = 128        # partitions
TB = T // P    # timesteps per partition block   (32)
TI = 8         # timesteps per chunk
NCH = TB // TI # chunks                          (4)
CW = TI * NF   # product/psum cols per chunk     (768)
CWU = TI * U   # out-tile cols per chunk         (1536)
JH = TB + FS - 1  # haloed time slots per row    (36)

_nc_cache = None


def _body(nc, tc, spec_d, coefs_d, ident_d, out_d):
    f32 = mybir.dt.float32
    mult = mybir.AluOpType.mult

    specv = spec_d.rearrange("(q i) u -> q i u", i=TB)   # [128, 32, 962]

    with (
        tc.tile_pool(name="const", bufs=1) as cpool,
        tc.tile_pool(name="spec", bufs=1) as spool,
        tc.tile_pool(name="coef", bufs=8) as kpool,
        tc.tile_pool(name="prod", bufs=6) as ppool,
        tc.tile_pool(name="de", bufs=6) as depool,
        tc.tile_pool(name="outp", bufs=3) as opool,
        tc.tile_pool(name="psum", bufs=2, space="PSUM") as pspool,
    ):
        ident_sb = cpool.tile([P, P], f32)
        nc.sync.dma_start(out=ident_sb[:], in_=ident_d)

        # hi-band passthrough: DRAM -> DRAM on the Scalar ring
        for c in range(4):
            r0, r1 = c * (T // 4), (c + 1) * (T // 4)
            nc.scalar.dma_start(out=out_d[r0:r1, U:ROW], in_=spec_d[r0:r1, U:ROW])

        # lo-band spec, haloed: row p free slot j in [0,36) holds t = 32p-4+j
        spec_raw = spool.tile([P, JH * U], f32)
        sv = spec_raw[:].rearrange("p (j u) -> p j u", u=U)      # [128, 36, 192]
        nc.vector.memzero(sv[0:1, 0:FS - 1, :])                   # t<0 -> 0
        nc.sync.dma_start(out=sv[:, FS - 1:JH, :], in_=specv[:, :, 0:U])
        nc.sync.dma_start(
            out=sv[1:P, 0:FS - 1, :],
            in_=specv[0:P - 1, TB - (FS - 1):TB, 0:U],
        )
        svfc = spec_raw[:].rearrange("p (j f c) -> p j f c", f=NF, c=2)

        # coefs per tap, chunked:  [NCH, 128, TI*U] with 6KB contiguous rows
        cviews = [
            coefs_d[k].rearrange("(q c i) u -> c q (i u)", c=NCH, i=TI)
            for k in range(FS)
        ]
        # output lo-band rows, chunked
        oview = out_d.rearrange("(q c i) u -> c q i u", c=NCH, i=TI)

        for ch in range(NCH):
            i0 = ch * TI
            ctiles = []
            for k in range(FS):
                ct = kpool.tile([P, TI * U], f32, tag="coef")
                nc.sync.dma_start(out=ct[:], in_=cviews[k][ch])
                ctiles.append(ct)

            ps_re = pspool.tile([P, CW], f32, tag="psre")
            ps_im = pspool.tile([P, CW], f32, tag="psim")

            for k in range(FS):
                s_re = svfc[:, i0 + k:i0 + k + TI, :, 0]          # [128, 8, 96]
                s_im = svfc[:, i0 + k:i0 + k + TI, :, 1]
                cvfc = ctiles[k][:].rearrange("p (i f c) -> p i f c", f=NF, c=2)
                c_re = cvfc[:, :, :, 0]
                c_im = cvfc[:, :, :, 1]

                prr = ppool.tile([P, CW], f32, tag="prod")
                pii = ppool.tile([P, CW], f32, tag="prod")
                pri = ppool.tile([P, CW], f32, tag="prod")
                pir = ppool.tile([P, CW], f32, tag="prod")
                dt_ = depool.tile([P, CW], f32, tag="de")
                et_ = depool.tile([P, CW], f32, tag="de")
                pv = lambda t: t[:].rearrange("p (i f) -> p i f", f=NF)

                nc.vector.tensor_mul(out=pv(prr), in0=s_re, in1=c_re)
                nc.vector.scalar_tensor_tensor(
                    out=pv(pii), in0=s_im, scalar=-1.0, in1=c_im,
                    op0=mult, op1=mult,
                )
                nc.vector.tensor_mul(out=pv(pri), in0=s_re, in1=c_im)
                nc.vector.tensor_mul(out=pv(pir), in0=s_im, in1=c_re)
                nc.vector.tensor_add(out=dt_[:], in0=prr[:], in1=pii[:])
                nc.vector.tensor_add(out=et_[:], in0=pri[:], in1=pir[:])

                # 5-tap accumulate in PSUM via identity matmul
                for src, ps in ((dt_, ps_re), (et_, ps_im)):
                    for a, b in ((0, 512), (512, CW)):
                        nc.tensor.matmul(
                            ps[:, a:b], ident_sb[:], src[:, a:b],
                            start=(k == 0), stop=(k == FS - 1),
                        )

            out_t = opool.tile([P, CWU], f32, tag="out")
            ov = out_t[:].rearrange("p (i f c) -> p i f c", f=NF, c=2)
            psv = lambda t: t[:].rearrange("p (i f) -> p i f", f=NF)
            nc.scalar.copy(out=ov[:, :, :, 0], in_=psv(ps_re))
            nc.scalar.copy(out=ov[:, :, :, 1], in_=psv(ps_im))
            nc.scalar.dma_start(
                out=oview[ch][:, :, 0:U],
                in_=out_t[:].rearrange("p (i u) -> p i u", u=U),
            )


def _build_nc():
    nc = bacc.Bacc("TRN2", target_bir_lowering=False, debug=False, num_devices=B)
    f32 = mybir.dt.float32
    spec_d = nc.dram_tensor("spec", [T, ROW], f32, kind="ExternalInput").ap()
    coefs_d = nc.dram_tensor("coefs", [FS, T, U], f32, kind="ExternalInput").ap()
    ident_d = nc.dram_tensor("ident", [P, P], f32, kind="ExternalInput").ap()
    out_d = nc.dram_tensor("out", [T, ROW], f32, kind="ExternalOutput").ap()
    with TileContext(nc) as tc:
        _body(nc, tc, spec_d, coefs_d, ident_d, out_d)
    nc.compile()
    return nc


def _in_maps(spec, coefs):
    spec = np.asarray(spec, dtype=np.float32)
    coefs = np.asarray(coefs, dtype=np.float32)
    ident = np.eye(P, dtype=np.float32)
    maps = []
    for b in range(B):
        maps.append({
            "spec": np.ascontiguousarray(spec[b, 0].reshape(T, ROW)),
            "coefs": np.ascontiguousarray(coefs[b].reshape(FS, T, U)),
            "ident": ident,
        })
    return maps


def kernel(spec, coefs):
    global _nc_cache
    if _nc_cache is None:
        _nc_cache = _build_nc()
    res = run_bass_kernel_spmd(_nc_cache, _in_maps(spec, coefs),
                               core_ids=list(range(B)))
    return np.stack(
        [res.results[b]["out"].reshape(1, T, F, 2) for b in range(B)]
    ).astype(np.float32)


# revision 3
# speedup vs baseline: 1.1188x; 1.0959x over previous
"""Trainium2 Bass kernel for the DfOp deep-filtering module.

out[b, t, f<96]  = sum_{k=0..4} coefs[b, k, t, f] (*) spec[b, t-4+k, f]   (complex mult)
out[b, t, f>=96] = spec[b, t, f]                                          (passthrough)

Sharding: data-parallel over batch B=8 -> one batch element per NeuronCore.

Per-core layout: partition p holds the 32-timestep block t in [32p, 32p+32),
processed in 4 chunks of 8 timesteps.  Spec is loaded as FULL 962-float DRAM
rows (one 30.8KB contiguous run per partition per chunk -> 128 descriptors
per DMA, near-peak HBM streaming).  The filtered lo-band is written back IN
PLACE into the same tile (the hi-band passthrough then never moves on-chip),
and the tile is stored back as full rows.  The causal 5-tap window reads
lo-band slices of the chunk tile via strided views; the 4 leading halo slots
come from a small halo tile extracted from the previous chunk (chunk 0's halo
is a tiny DMA + memset for t<0).

Compute (all fp32, bit-exact accumulation):
  DVE: per tap, 4 real products via strided access patterns (fp32
       tensor_tensor is 1x-mode regardless of stride), the im*im negation
       fused via scalar_tensor_tensor; then pair-combines D = rr - ii,
       E = ri + ir (in place over the rr/ri product tiles).
  PE : accumulates the 5 taps' D (resp. E) into PSUM with identity-weight
       matmuls (exact fp32 PSUM accumulate).
  ACT: interleaves PSUM re/im into the chunk tile's lo-band, extracts halos.
  DMA: loads on the Sync HWDGE ring, stores on the Scalar HWDGE ring.
"""

import numpy as np

import concourse.bacc as bacc
import concourse.mybir as mybir
from concourse.tile import TileContext
from concourse.bass_utils import run_bass_kernel_spmd

B = 8          # batch / cores
T = 4096       # time steps
F = 481        # total freq bins
NF = 96        # deep-filtered freq bins
FS = 5         # frame size (causal taps)
HL = FS - 1    # halo slots (4)
ROW = 2 * F    # floats per DRAM time row        (962)
U = 2 * NF     # lo-band floats per time row     (192)
P = 128        # partitions
TB = T // P    # timesteps per partition block   (32)
TI = 8         # timesteps per chunk
NCH = TB // TI # chunks                          (4)
CW = TI * NF   # product/psum cols per chunk     (768)
CROW = TI * ROW  # full-row chunk cols           (7696)

_nc_cache = None


def _body(nc, tc, spec_d, coefs_d, ident_d, out_d):
    f32 = mybir.dt.float32
    mult = mybir.AluOpType.mult

    specv = spec_d.rearrange("(q i) u -> q i u", i=TB)          # [128, 32, 962]
    scv = spec_d.rearrange("(q c i) u -> c q (i u)", c=NCH, i=TI)
    ocv = out_d.rearrange("(q c i) u -> c q (i u)", c=NCH, i=TI)
    cviews = [
        coefs_d[k].rearrange("(q c i) u -> c q (i u)", c=NCH, i=TI)
        for k in range(FS)
    ]

    with (
        tc.tile_pool(name="const", bufs=1) as cpool,
        tc.tile_pool(name="spec", bufs=3) as spool,
        tc.tile_pool(name="halo", bufs=3) as hpool,
        tc.tile_pool(name="coef", bufs=8) as kpool,
        tc.tile_pool(name="prod", bufs=6) as ppool,
        tc.tile_pool(name="psum", bufs=2, space="PSUM") as pspool,
    ):
        ident_sb = cpool.tile([P, P], f32)
        nc.sync.dma_start(out=ident_sb[:], in_=ident_d)

        # chunk-0 halo: t in [32p-4, 32p), zeros for t<0
        halo = hpool.tile([P, HL * U], f32, tag="halo")
        nc.vector.memzero(halo[0:1, :])
        nc.sync.dma_start(
            out=halo[:].rearrange("p (j u) -> p j u", u=U)[1:P],
            in_=specv[0:P - 1, TB - HL:TB, 0:U],
        )

        for ch in range(NCH):
            stile = spool.tile([P, CROW], f32, tag="spec")
            nc.sync.dma_start(out=stile[:], in_=scv[ch])
            ctiles = []
            for k in range(FS):
                ct = kpool.tile([P, TI * U], f32, tag="coef")
                nc.sync.dma_start(out=ct[:], in_=cviews[k][ch])
                ctiles.append(ct)

            sfc = stile[:].rearrange("p (i f c) -> p i f c", f=F, c=2)
            hfc = halo[:].rearrange("p (j f c) -> p j f c", f=NF, c=2)

            # extract next chunk's halo (last 4 slots' lo band) before the
            # in-place lo-band overwrite below
            if ch < NCH - 1:
                nhalo = hpool.tile([P, HL * U], f32, tag="halo")
                nc.scalar.copy(
                    out=nhalo[:].rearrange("p (j f c) -> p j f c", f=NF, c=2),
                    in_=sfc[:, TI - HL:TI, 0:NF, :],
                )

            ps_re = pspool.tile([P, CW], f32, tag="psre")
            ps_im = pspool.tile([P, CW], f32, tag="psim")

            for k in range(FS):
                nh = HL - k                # output slots fed from the halo tile
                cvfc = ctiles[k][:].rearrange("p (i f c) -> p i f c", f=NF, c=2)
                prr = ppool.tile([P, CW], f32, tag="prod")
                pii = ppool.tile([P, CW], f32, tag="prod")
                pri = ppool.tile([P, CW], f32, tag="prod")
                pir = ppool.tile([P, CW], f32, tag="prod")
                pv = lambda t: t[:].rearrange("p (i f) -> p i f", f=NF)

                # (s_src, c_slice, out_slice) pieces: halo part + main part
                pieces = []
                if nh > 0:
                    pieces.append((
                        lambda c, k=k, nh=nh: hfc[:, k:HL, :, c],
                        lambda ap, nh=nh: ap[:, 0:nh],
                    ))
                pieces.append((
                    lambda c, k=k, nh=nh: sfc[:, 0:TI - nh, 0:NF, c],
                    lambda ap, nh=nh: ap[:, nh:TI],
                ))

                for s_src, sl in pieces:
                    s_re, s_im = s_src(0), s_src(1)
                    c_re = sl(cvfc)[:, :, :, 0]
                    c_im = sl(cvfc)[:, :, :, 1]
                    nc.vector.tensor_mul(out=sl(pv(prr)), in0=s_re, in1=c_re)
                    nc.vector.scalar_tensor_tensor(
                        out=sl(pv(pii)), in0=s_im, scalar=-1.0, in1=c_im,
                        op0=mult, op1=mult,
                    )
                    nc.vector.tensor_mul(out=sl(pv(pri)), in0=s_re, in1=c_im)
                    nc.vector.tensor_mul(out=sl(pv(pir)), in0=s_im, in1=c_re)

                nc.vector.tensor_add(out=prr[:], in0=prr[:], in1=pii[:])  # D
                nc.vector.tensor_add(out=pri[:], in0=pri[:], in1=pir[:])  # E

                for src, ps in ((prr, ps_re), (pri, ps_im)):
                    for a, b in ((0, 512), (512, CW)):
                        nc.tensor.matmul(
                            ps[:, a:b], ident_sb[:], src[:, a:b],
                            start=(k == 0), stop=(k == FS - 1),
                        )

            # interleave PSUM into the tile's lo band (in place), store rows
            psv = lambda t: t[:].rearrange("p (i f) -> p i f", f=NF)
            nc.scalar.copy(out=sfc[:, :, 0:NF, 0], in_=psv(ps_re))
            nc.scalar.copy(out=sfc[:, :, 0:NF, 1], in_=psv(ps_im))
            nc.scalar.dma_start(out=ocv[ch], in_=stile[:])

            if ch < NCH - 1:
                halo = nhalo


def _build_nc():
    nc = bacc.Bacc("TRN2", target_bir_lowering=False, debug=False, num_devices=B)
    f32 = mybir.dt.float32
    spec_d = nc.dram_tensor("spec", [T, ROW], f32, kind="ExternalInput").ap()
    coefs_d = nc.dram_tensor("coefs", [FS, T, U], f32, kind="ExternalInput").ap()
    ident_d = nc.dram_tensor("ident", [P, P], f32, kind="ExternalInput").ap()
    out_d = nc.dram_tensor("out", [T, ROW], f32, kind="ExternalOutput").ap()
    with TileContext(nc) as tc:
        _body(nc, tc, spec_d, coefs_d, ident_d, out_d)
    nc.compile()
    return nc


def _in_maps(spec, coefs):
    spec = np.asarray(spec, dtype=np.float32)
    coefs = np.asarray(coefs, dtype=np.float32)
    ident = np.eye(P, dtype=np.float32)
    maps = []
    for b in range(B):
        maps.append({
            "spec": np.ascontiguousarray(spec[b, 0].reshape(T, ROW)),
            "coefs": np.ascontiguousarray(coefs[b].reshape(FS, T, U)),
            "ident": ident,
        })
    return maps


def kernel(spec, coefs):
    global _nc_cache
    if _nc_cache is None:
        _nc_cache = _build_nc()
    res = run_bass_kernel_spmd(_nc_cache, _in_maps(spec, coefs),
                               core_ids=list(range(B)))
    return np.stack(
        [res.results[b]["out"].reshape(1, T, F, 2) for b in range(B)]
    ).astype(np.float32)


# revision 4
# speedup vs baseline: 1.1767x; 1.0517x over previous
"""Trainium2 Bass kernel for the DfOp deep-filtering module.

out[b, t, f<96]  = sum_{k=0..4} coefs[b, k, t, f] (*) spec[b, t-4+k, f]   (complex mult)
out[b, t, f>=96] = spec[b, t, f]                                          (passthrough)

Sharding: data-parallel over batch B=8 -> one batch element per NeuronCore.

Per-core layout: partition p holds the 32-timestep block t in [32p, 32p+32),
processed in chunks of [4, 10, 10, 8] timesteps (small first chunk so the
compute pipeline starts early).  Spec is loaded as FULL 962-float DRAM rows
(one contiguous run per partition per chunk -> 128 descriptors per DMA,
near-peak HBM streaming).  The filtered lo-band is written back IN PLACE
into the same tile (the hi-band passthrough then never moves on-chip) and
the tile is stored back as full rows.

Each chunk materializes a packed "window" tile holding the chunk's lo-band
plus a 4-slot halo (chained from the previous chunk's window), so the causal
5-tap window is a pure free-dim offset and every DVE product is one unsplit
instruction.

Compute (all fp32, bit-exact accumulation):
  DVE: per tap, 4 real products (rr, -ii via fused scalar_tensor_tensor,
       ri, ir) + pair-combines D = rr - ii, E = ri + ir.
  PE : accumulates the 5 taps' D (resp. E) into PSUM with identity-weight
       matmuls (exact fp32 PSUM accumulate).
  ACT: window fills/extracts, PSUM->lo-band interleave.
  DMA: loads + last-chunk hi-band store on the Sync HWDGE ring; row stores
       on the Scalar HWDGE ring.
"""

import numpy as np

import concourse.bacc as bacc
import concourse.mybir as mybir
from concourse.tile import TileContext
from concourse.bass_utils import run_bass_kernel_spmd

B = 8          # batch / cores
T = 4096       # time steps
F = 481        # total freq bins
NF = 96        # deep-filtered freq bins
FS = 5         # frame size (causal taps)
HL = FS - 1    # halo slots (4)
ROW = 2 * F    # floats per DRAM time row        (962)
U = 2 * NF     # lo-band floats per time row     (192)
P = 128        # partitions
TB = T // P    # timesteps per partition block   (32)
SIZES = [4, 10, 10, 8]        # per-chunk timesteps (sum = TB)
OFFS = [0, 4, 14, 24]         # cumulative offsets
WMAX = (max(SIZES) + HL) * U  # window tile cols

_nc_cache = None


def _mm_ranges(cw):
    return [(a, min(a + 512, cw)) for a in range(0, cw, 512)]


def _body(nc, tc, spec_d, coefs_d, ident_d, out_d):
    f32 = mybir.dt.float32
    mult = mybir.AluOpType.mult

    specv = spec_d.rearrange("(q i) u -> q i u", i=TB)          # [128, 32, 962]
    outv = out_d.rearrange("(q i) u -> q i u", i=TB)
    coefv = [coefs_d[k].rearrange("(q i) u -> q i u", i=TB) for k in range(FS)]

    with (
        tc.tile_pool(name="const", bufs=1) as cpool,
        tc.tile_pool(name="spec", bufs=2) as spool,
        tc.tile_pool(name="win", bufs=2) as wpool,
        tc.tile_pool(name="coef", bufs=7) as kpool,
        tc.tile_pool(name="prod", bufs=5) as ppool,
        tc.tile_pool(name="psum", bufs=2, space="PSUM") as pspool,
    ):
        ident_sb = cpool.tile([P, P], f32)
        nc.sync.dma_start(out=ident_sb[:], in_=ident_d)

        prev_w = None
        prev_ti = None
        for ch, (i0, TI) in enumerate(zip(OFFS, SIZES)):
            CW = TI * NF

            stile = spool.tile([P, TI * ROW], f32, tag="spec")
            nc.sync.dma_start(
                out=stile[:],
                in_=specv[:, i0:i0 + TI, :].rearrange("q i u -> q (i u)"),
            )
            ctiles = []
            for k in range(FS):
                ct = kpool.tile([P, TI * U], f32, tag="coef")
                nc.sync.dma_start(
                    out=ct[:],
                    in_=coefv[k][:, i0:i0 + TI, :].rearrange("q i u -> q (i u)"),
                )
                ctiles.append(ct)

            # window tile: [halo(4) | chunk lo-band(TI)] packed, 192 floats/slot
            wtile = wpool.tile([P, WMAX], f32, tag="win")
            if ch == 0:
                nc.vector.memzero(wtile[0:1, 0:HL * U])
                nc.sync.dma_start(
                    out=wtile[:].rearrange("p (j u) -> p j u", u=U)[1:P, 0:HL],
                    in_=specv[0:P - 1, TB - HL:TB, 0:U],
                )
            else:
                nc.scalar.copy(
                    out=wtile[:, 0:HL * U],
                    in_=prev_w[:, prev_ti * U:(prev_ti + HL) * U],
                )
            sfc = stile[:].rearrange("p (i f c) -> p i f c", f=F, c=2)
            nc.scalar.copy(
                out=wtile[:].rearrange("p (j u) -> p j u", u=U)[:, HL:HL + TI],
                in_=sfc[:, :, 0:NF, :].rearrange("p i f c -> p i (f c)"),
            )
            wfc = wtile[:].rearrange("p (j f c) -> p j f c", f=NF, c=2)

            # last chunk: store the untouched hi-band early (overlaps compute),
            # so the final row store only covers the lo-band
            if ch == len(SIZES) - 1:
                nc.sync.dma_start(
                    out=outv[:, i0:i0 + TI, U:ROW],
                    in_=sfc[:, :, NF:F, :].rearrange("p i f c -> p i (f c)"),
                )

            ps_re = pspool.tile([P, CW], f32, tag="psre")
            ps_im = pspool.tile([P, CW], f32, tag="psim")

            for k in range(FS):
                s_re = wfc[:, k:k + TI, :, 0]                 # [128, TI, 96]
                s_im = wfc[:, k:k + TI, :, 1]
                cvfc = ctiles[k][:].rearrange("p (i f c) -> p i f c", f=NF, c=2)
                c_re = cvfc[:, :, :, 0]
                c_im = cvfc[:, :, :, 1]

                prr = ppool.tile([P, CW], f32, tag="prod")
                pii = ppool.tile([P, CW], f32, tag="prod")
                pri = ppool.tile([P, CW], f32, tag="prod")
                pir = ppool.tile([P, CW], f32, tag="prod")
                pv = lambda t: t[:].rearrange("p (i f) -> p i f", f=NF)

                nc.vector.tensor_mul(out=pv(prr), in0=s_re, in1=c_re)
                nc.vector.scalar_tensor_tensor(
                    out=pv(pii), in0=s_im, scalar=-1.0, in1=c_im,
                    op0=mult, op1=mult,
                )
                nc.vector.tensor_mul(out=pv(pri), in0=s_re, in1=c_im)
                nc.vector.tensor_mul(out=pv(pir), in0=s_im, in1=c_re)
                nc.vector.tensor_add(out=prr[:], in0=prr[:], in1=pii[:])  # D
                nc.vector.tensor_add(out=pri[:], in0=pri[:], in1=pir[:])  # E

                for src, ps in ((prr, ps_re), (pri, ps_im)):
                    for a, b in _mm_ranges(CW):
                        nc.tensor.matmul(
                            ps[:, a:b], ident_sb[:], src[:, a:b],
                            start=(k == 0), stop=(k == FS - 1),
                        )

            # interleave PSUM into the tile's lo band (in place), store rows
            psv = lambda t: t[:].rearrange("p (i f) -> p i f", f=NF)
            nc.scalar.copy(out=sfc[:, :, 0:NF, 0], in_=psv(ps_re))
            nc.scalar.copy(out=sfc[:, :, 0:NF, 1], in_=psv(ps_im))
            if ch == len(SIZES) - 1:
                nc.scalar.dma_start(
                    out=outv[:, i0:i0 + TI, 0:U],
                    in_=sfc[:, :, 0:NF, :].rearrange("p i f c -> p i (f c)"),
                )
            else:
                nc.scalar.dma_start(
                    out=outv[:, i0:i0 + TI, :].rearrange("q i u -> q (i u)"),
                    in_=stile[:],
                )

            prev_w, prev_ti = wtile, TI


def _build_nc():
    nc = bacc.Bacc("TRN2", target_bir_lowering=False, debug=False, num_devices=B)
    f32 = mybir.dt.float32
    spec_d = nc.dram_tensor("spec", [T, ROW], f32, kind="ExternalInput").ap()
    coefs_d = nc.dram_tensor("coefs", [FS, T, U], f32, kind="ExternalInput").ap()
    ident_d = nc.dram_tensor("ident", [P, P], f32, kind="ExternalInput").ap()
    out_d = nc.dram_tensor("out", [T, ROW], f32, kind="ExternalOutput").ap()
    with TileContext(nc) as tc:
        _body(nc, tc, spec_d, coefs_d, ident_d, out_d)
    nc.compile()
    return nc


def _in_maps(spec, coefs):
    spec = np.asarray(spec, dtype=np.float32)
    coefs = np.asarray(coefs, dtype=np.float32)
    ident = np.eye(P, dtype=np.float32)
    maps = []
    for b in range(B):
        maps.append({
            "spec": np.ascontiguousarray(spec[b, 0].reshape(T, ROW)),
            "coefs": np.ascontiguousarray(coefs[b].reshape(FS, T, U)),
            "ident": ident,
        })
    return maps


def kernel(spec, coefs):
    global _nc_cache
    if _nc_cache is None:
        _nc_cache = _build_nc()
    res = run_bass_kernel_spmd(_nc_cache, _in_maps(spec, coefs),
                               core_ids=list(range(B)))
    return np.stack(
        [res.results[b]["out"].reshape(1, T, F, 2) for b in range(B)]
    ).astype(np.float32)
